# revision 4
# baseline (speedup 1.0000x reference)
"""Trainium2 Bass kernel for nn_Encoder (MHA encoder block) — v3.

Problem: x (2, 2048, 1024) fp32; per-head attention (16 heads x 64) with
QKV/O projections + biases; softmax WITHOUT 1/sqrt(hs) scaling.

Sharding (8 cores): core c handles batch n = c//4 and a group of 4 heads
hg = c%4 (features fs = 256*hg .. +256).

v3 design:
  - 16-bit matmul operands everywhere: Q/K path fp16 (10-bit mantissa; bf16's
    8 bits fail the 2e-2 gate since logit errors scale with |S| <= 76), V/E/
    C~/Wo bf16 (E = exp(S) up to e^70 overflows fp16's range).  16-bit
    weights enable FWL weight loads + LDW pull-ahead on PE (measured 217 ns
    per K=128 N=512 MM vs 230-258 fp32r).
  - S matmul head pairs carry explicit tile_position (0,0)/(64,0): the two
    K=64 MMs run concurrently in disjoint PE row groups (measured 234 ns per
    pair vs 320 fp32r / 640 serial).
  - x resides in SBUF (fp16, 4.2 MB), so projections read it directly.
  - Schedule: KT/V projection for key-block tb is interleaved into query
    block 0's attention stream (S(qb0, kc) only needs KT/V chunk kc), and QT
    projection is deferred per query block; after the first ~11 us the kernel
    is a single continuous stream with ACT (exp, ~142 us) and PE (~143 us)
    both near-saturated.
  - exp batches head pairs: ACTIVATE on [128, 2, 512] PSUM tiles (FD 1024).
  - Denominators ride row 64 of the AV matmul (V augmented with ones col);
    normalization via reciprocal + selector-matmul broadcast + DVE multiply.
"""

import numpy as np

HIDDEN = 1024
HEADS = 16
HS = 64
L = 2048
NB = 2
NCORES = 8
HPC = 4          # heads per core
F = HPC * HS     # 256 per-core head features
KC = HIDDEN // 128   # 8 hidden chunks
TB = L // 512        # 4 token blocks of 512
TC = L // 128        # 16 token chunks of 128
KCH = L // 128       # 16 key chunks of 128

_CACHE = {}


def round_fp32r(a: np.ndarray) -> np.ndarray:
    """Round fp32 to the fp32r encoding (12-bit mantissa, round half up)."""
    bits = np.ascontiguousarray(a, dtype=np.float32).view(np.uint32)
    r = ((bits.astype(np.uint64) + 0x800) & 0xFFFFF000).astype(np.uint32)
    return r.view(np.float32)


def _sel_matrix():
    sel = np.zeros((HPC, 2, 128), dtype=np.float32)
    for chunk in range(2):
        for j in range(2):
            sel[2 * chunk + j, chunk, 64 * j:64 * j + 64] = 1.0
    return sel


def to_bf16_np(a):
    import ml_dtypes
    return np.ascontiguousarray(
        np.ascontiguousarray(a, dtype=np.float32).astype(ml_dtypes.bfloat16)
    )


def _build(loop_n: int = 1):
    import concourse.mybir as mybir
    import concourse.tile as tile
    from concourse import bacc

    F32 = mybir.dt.float32
    F32R = mybir.dt.float32r
    F16 = mybir.dt.float16
    BF16 = mybir.dt.bfloat16
    AF = mybir.ActivationFunctionType

    nc = bacc.Bacc("TRN2", target_bir_lowering=False, debug=False)

    xT = nc.dram_tensor("xT", [128, KC, L], F16, kind="ExternalInput")
    wq = nc.dram_tensor("wq", [128, KC, F], F16, kind="ExternalInput")
    wk = nc.dram_tensor("wk", [128, KC, F], F16, kind="ExternalInput")
    wv = nc.dram_tensor("wv", [128, KC, F], F16, kind="ExternalInput")
    wo = nc.dram_tensor("wo", [128, 2, HIDDEN], BF16, kind="ExternalInput")
    bq = nc.dram_tensor("bq", [128, 2], F32, kind="ExternalInput")
    bk = nc.dram_tensor("bk", [128, 2], F32, kind="ExternalInput")
    bv = nc.dram_tensor("bv", [1, F], F32, kind="ExternalInput")
    sel = nc.dram_tensor("sel", [HPC, 2, 128], F32R, kind="ExternalInput")
    po = nc.dram_tensor("po", [128, TC, HIDDEN], F32, kind="ExternalOutput")

    with tile.TileContext(nc) as tc:
        with (
            tc.tile_pool(name="const", bufs=1) as const,
            tc.tile_pool(name="work", bufs=2) as work,
            tc.tile_pool(name="es", bufs=4) as es,
            tc.tile_pool(name="pout", bufs=3) as pout,
            # PSUM budget (8 banks): s 2x2 + cacc 2 + mm 2 (proj/po/bcast)
            tc.tile_pool(name="ps_mm", bufs=2, space="PSUM") as ps_mm,
            tc.tile_pool(name="ps_s", bufs=2, space="PSUM") as ps_s,
            tc.tile_pool(name="ps_c", bufs=2, space="PSUM") as ps_c,
        ):
            # ---------------- persistent tiles + one-time input DMA ----------
            # DMA order = need order: the first MMs touch x(tb0), wk, bk.
            x_sb = const.tile([128, KC, L], F16)

            def x_dma(tb):
                # one descriptor-set per block: stripes across all 16 HW DMA
                # queues automatically, avoids per-dma_start SWDGE startup
                nc.sync.dma_start(
                    x_sb[:, :, tb * 512:(tb + 1) * 512],
                    xT.ap()[:, :, tb * 512:(tb + 1) * 512],
                )

            x_dma(0)
            wk_sb = const.tile([128, KC, F], F16)
            nc.sync.dma_start(wk_sb, wk.ap())
            bk_sb = const.tile([128, 2], F32)
            nc.sync.dma_start(bk_sb, bk.ap())
            wv_sb = const.tile([128, KC, F], F16)
            nc.sync.dma_start(wv_sb, wv.ap())
            bv_row = const.tile([1, F], F32)
            nc.sync.dma_start(bv_row, bv.ap())
            wq_sb = const.tile([128, KC, F], F16)
            nc.sync.dma_start(wq_sb, wq.ap())
            bq_sb = const.tile([128, 2], F32)
            nc.sync.dma_start(bq_sb, bq.ap())
            sel_r = const.tile([HPC, 2, 128], F32R)
            nc.sync.dma_start(sel_r, sel.ap())
            wo_sb = const.tile([128, 2, HIDDEN], BF16)
            nc.sync.dma_start(wo_sb, wo.ap())
            for tb in range(1, TB):
                x_dma(tb)
            # V bias broadcast across all 128 token partitions (Pool engine)
            bv_sb = const.tile([128, F], F32)
            nc.gpsimd.partition_broadcast(bv_sb, bv_row, channels=128)

            qt_sb = const.tile([128, 2, L], F16)   # [feat%128, feat//128, q]
            kt_sb = const.tile([128, 2, L], F16)
            # V augmented with a ones column per head: [tok%128, tok//128, h, 65]
            v_sb = const.tile([128, TC, HPC, HS + 1], BF16)
            # C~^T, normalized in place later: [feat%128, feat//128, q]
            c_sb = const.tile([128, 2, L], BF16)
            # softmax denominators [h, qb, 512] and their f32r reciprocals
            d_sb = const.tile([HPC, TB, 512], F32)
            rr_sb = const.tile([HPC, TB, 512], F32R)

            onecol_f = const.tile([128, 1], F32)
            nc.vector.memset(onecol_f, 1.0)
            nc.vector.tensor_copy(
                v_sb[:, :, :, HS:HS + 1],
                onecol_f.to_broadcast((128, TC, HPC, 1)),
            )

            def qt_proj(qb):
                for fc in range(2):
                    pt = ps_mm.tile([128, 512], F32, tag="mm")
                    for kc in range(KC):
                        nc.tensor.matmul(
                            pt,
                            wq_sb[:, kc, fc * 128:(fc + 1) * 128],
                            x_sb[:, kc, qb * 512:(qb + 1) * 512],
                            start=(kc == 0),
                            stop=(kc == KC - 1),
                        )
                    nc.vector.tensor_scalar(
                        qt_sb[:, fc, qb * 512:(qb + 1) * 512],
                        pt,
                        bq_sb[:, fc:fc + 1],
                        None,
                        mybir.AluOpType.add,
                    )

            def kt_piece(tb, fc):
                pt = ps_mm.tile([128, 512], F32, tag="mm")
                for kc in range(KC):
                    nc.tensor.matmul(
                        pt,
                        wk_sb[:, kc, fc * 128:(fc + 1) * 128],
                        x_sb[:, kc, tb * 512:(tb + 1) * 512],
                        start=(kc == 0),
                        stop=(kc == KC - 1),
                    )
                nc.vector.tensor_scalar(
                    kt_sb[:, fc, tb * 512:(tb + 1) * 512],
                    pt,
                    bk_sb[:, fc:fc + 1],
                    None,
                    mybir.AluOpType.add,
                )

            def v_piece(tb, half):
                for sub in (2 * half, 2 * half + 1):
                    t16 = tb * 4 + sub
                    pv = ps_mm.tile([128, 512], F32, tag="mm")
                    for kc in range(KC):
                        nc.tensor.matmul(
                            pv[:, :F],
                            x_sb[:, kc, t16 * 128:(t16 + 1) * 128],
                            wv_sb[:, kc, :],
                            start=(kc == 0),
                            stop=(kc == KC - 1),
                        )
                    nc.vector.tensor_tensor(
                        v_sb[:, t16, :, 0:HS],
                        pv[:, :F].rearrange("p (h s) -> p h s", h=HPC),
                        bv_sb.rearrange("p (h s) -> p h s", h=HPC),
                        mybir.AluOpType.add,
                    )

            def ktv_proj(tb):
                for fc in range(2):
                    pt = ps_mm.tile([128, 512], F32, tag="mm")
                    for kc in range(KC):
                        nc.tensor.matmul(
                            pt,
                            wk_sb[:, kc, fc * 128:(fc + 1) * 128],
                            x_sb[:, kc, tb * 512:(tb + 1) * 512],
                            start=(kc == 0),
                            stop=(kc == KC - 1),
                        )
                    nc.vector.tensor_scalar(
                        kt_sb[:, fc, tb * 512:(tb + 1) * 512],
                        pt,
                        bk_sb[:, fc:fc + 1],
                        None,
                        mybir.AluOpType.add,
                    )
                # V: out[t, f] on token partitions; bias added in the drain
                for sub in range(4):
                    t16 = tb * 4 + sub
                    pv = ps_mm.tile([128, 512], F32, tag="mm")
                    for kc in range(KC):
                        nc.tensor.matmul(
                            pv[:, :F],
                            x_sb[:, kc, t16 * 128:(t16 + 1) * 128],
                            wv_sb[:, kc, :],
                            start=(kc == 0),
                            stop=(kc == KC - 1),
                        )
                    nc.vector.tensor_tensor(
                        v_sb[:, t16, :, 0:HS],
                        pv[:, :F].rearrange("p (h s) -> p h s", h=HPC),
                        bv_sb.rearrange("p (h s) -> p h s", h=HPC),
                        mybir.AluOpType.add,
                    )

            def attention(qb, interleave=None):
                """S -> exp -> AV for one query block, software-pipelined in
                GROUPS of two key chunks: the 4 S matmuls of a group are
                emitted back-to-back (a pure S stream row-pipelines in the PE
                array at ~283 ns/pair; interleaving AV matmuls between pairs
                breaks that and costs ~641 ns/pair), then the group's two
                exps, then the previous group's 4 AV matmuls.

                interleave: optional dict {group: fn} of work to emit before
                group g of hp0 (used to stream KT/V projection of later key
                blocks into qb0's attention and the previous block's
                normalize/out-proj into this one).
                """
                NG = KCH // 2
                for hp in range(2):
                    ha, hb = 2 * hp, 2 * hp + 1
                    cacc_a = ps_c.tile([65, 512], F32, tag="cacc")
                    cacc_b = ps_c.tile([65, 512], F32, tag="cacc")
                    cacc = {ha: cacc_a, hb: cacc_b}

                    def av_block(g):
                        for kc in (2 * g, 2 * g + 1):
                            prev = ets.pop(kc)
                            for i, h in ((0, ha), (1, hb)):
                                nc.tensor.matmul(
                                    cacc[h],
                                    v_sb[:, kc, h, :],
                                    prev[:, i, :],
                                    start=(kc == 0),
                                    stop=(kc == KCH - 1),
                                )

                    ets = {}
                    for g in range(NG):
                        if interleave and (hp, g) in interleave:
                            interleave[(hp, g)]()
                        sps = {}
                        for kc in (2 * g, 2 * g + 1):
                            sp2 = ps_s.tile([128, 2, 512], F32, tag="s")
                            sps[kc] = sp2
                            for i, hr in ((0, 0), (1, 64)):
                                nc.tensor.matmul(
                                    sp2[:, i, :],
                                    kt_sb[hr:hr + 64, hp,
                                          kc * 128:(kc + 1) * 128],
                                    qt_sb[hr:hr + 64, hp,
                                          qb * 512:(qb + 1) * 512],
                                    start=True,
                                    stop=True,
                                    tile_position=(hr, 0),
                                )
                        for kc in (2 * g, 2 * g + 1):
                            et2 = es.tile([128, 2, 512], BF16, tag="e")
                            nc.scalar.activation(et2, sps[kc], AF.Exp)
                            ets[kc] = et2
                        if g >= 1:
                            av_block(g - 1)
                    av_block(NG - 1)
                    # C~^T rows -> c_sb; denominator row 64 -> staging, then a
                    # small DMA moves it across partitions.
                    for h, hr in ((ha, 0), (hb, 64)):
                        nc.vector.tensor_copy(
                            c_sb[hr:hr + 64, hp, qb * 512:(qb + 1) * 512],
                            cacc[h][0:64, :],
                        )
                        st = work.tile([65, 512], F32, tag="srow")
                        nc.vector.tensor_copy(st[64:65, :], cacc[h][64:65, :])
                        nc.sync.dma_start(d_sb[h:h + 1, qb, :], st[64:65, :])

            def normalize_chunk(qb, hp):
                # per-chunk variant for the final block's tail.  DVE slices
                # must start at partition 0, so chunk 1 re-processes all 4
                # rows: rows 0-1 get 1/(1/d) - finite garbage that the sel
                # slice zeroes out of the broadcast matmul.
                lo, n = (0, 2) if hp == 0 else (0, 4)
                nc.vector.reciprocal_approx_fast(
                    d_sb[lo:n, qb, :], d_sb[lo:n, qb, :],
                )
                nc.vector.tensor_copy(rr_sb[lo:n, qb, :], d_sb[lo:n, qb, :])
                bp = ps_mm.tile([128, 512], F32, tag="mm")
                nc.tensor.matmul(
                    bp, sel_r[:, hp, :], rr_sb[:, qb, :],
                    start=True, stop=True,
                )
                sl = c_sb[:, hp, qb * 512:(qb + 1) * 512]
                nc.vector.tensor_tensor(sl, sl, bp, mybir.AluOpType.mult)

            def po_proj(qb):
                for sub in range(4):
                    t16 = qb * 4 + sub
                    for jb in range(2):
                        pp = ps_mm.tile([128, 512], F32, tag="mm")
                        for chunk in range(2):
                            nc.tensor.matmul(
                                pp,
                                c_sb[:, chunk, t16 * 128:(t16 + 1) * 128],
                                wo_sb[:, chunk, jb * 512:(jb + 1) * 512],
                                start=(chunk == 0),
                                stop=(chunk == 1),
                            )
                        ot = pout.tile([128, 512], F32, tag="po")
                        nc.vector.tensor_copy(ot, pp)
                        nc.sync.dma_start(
                            po.ap()[:, t16, jb * 512:(jb + 1) * 512], ot
                        )

            def normalize_po(qb):
                nc.vector.reciprocal_approx_fast(d_sb[:, qb, :], d_sb[:, qb, :])
                nc.vector.tensor_copy(rr_sb[:, qb, :], d_sb[:, qb, :])
                for chunk in range(2):
                    bp = ps_mm.tile([128, 512], F32, tag="mm")
                    nc.tensor.matmul(
                        bp, sel_r[:, chunk, :], rr_sb[:, qb, :],
                        start=True, stop=True,
                    )
                    sl = c_sb[:, chunk, qb * 512:(qb + 1) * 512]
                    nc.vector.tensor_tensor(sl, sl, bp, mybir.AluOpType.mult)
                for sub in range(4):
                    t16 = qb * 4 + sub
                    for jb in range(2):
                        pp = ps_mm.tile([128, 512], F32, tag="mm")
                        for chunk in range(2):
                            nc.tensor.matmul(
                                pp,
                                c_sb[:, chunk, t16 * 128:(t16 + 1) * 128],
                                wo_sb[:, chunk, jb * 512:(jb + 1) * 512],
                                start=(chunk == 0),
                                stop=(chunk == 1),
                            )
                        ot = pout.tile([128, 512], F32, tag="po")
                        nc.vector.tensor_copy(ot, pp)
                        nc.sync.dma_start(
                            po.ap()[:, t16, jb * 512:(jb + 1) * 512], ot
                        )

            def body(_iv=None):
                if loop_n > 1:
                    for tb in range(TB):
                        x_dma(tb)
                kt_piece(0, 0)
                v_piece(0, 0)
                v_piece(0, 1)
                qt_proj(0)

                # qb0 attention streams the remaining key blocks' projections
                # in small pieces timed to when the S/AV stream needs them.
                attention(0, interleave={
                    (0, 1): lambda: kt_piece(1, 0),
                    (0, 2): lambda: v_piece(1, 0),
                    (0, 3): (lambda: (v_piece(1, 1), kt_piece(2, 0))),
                    (0, 4): lambda: v_piece(2, 0),
                    (0, 5): (lambda: (v_piece(2, 1), kt_piece(3, 0))),
                    (0, 6): (lambda: (v_piece(3, 0), kt_piece(0, 1))),
                    (0, 7): lambda: v_piece(3, 1),
                    (1, 0): lambda: kt_piece(1, 1),
                    (1, 2): lambda: kt_piece(2, 1),
                    (1, 4): lambda: kt_piece(3, 1),
                })
                # normalize+out-proj run one query block behind attention so
                # the denominator DMA/reciprocal latency hides under the next
                # block's S/exp/AV stream instead of stalling the PE queue.
                for qb in range(1, TB):
                    qt_proj(qb)
                    il = {(0, 1): (lambda q: lambda: normalize_po(q))(qb - 1)}
                    if qb == TB - 1:
                        il[(1, 2)] = lambda: normalize_chunk(TB - 1, 0)
                    attention(qb, interleave=il)
                normalize_chunk(TB - 1, 1)
                po_proj(TB - 1)

            if loop_n > 1:
                # body is ~850 PE instructions (> one IRAM block): hint the
                # back-edge so the branch target prefetches (~4us/iter saved)
                with tc.For_i(0, loop_n, 1,
                              hint_engines=(mybir.EngineType.PE,
                                            mybir.EngineType.Activation,
                                            mybir.EngineType.DVE,
                                            mybir.EngineType.SP)) as _i:
                    body(_i)
            else:
                body()

    nc.finalize()
    return nc


def _get_nc():
    if "nc" not in _CACHE:
        _CACHE["nc"] = _build()
    return _CACHE["nc"]


def _make_in_maps(x, Wq, bq, Wk, bk, Wv, bv, Wo):
    # per-batch xT in device layout [p, kc, t], fp16
    xTs = []
    for n in range(NB):
        xt = x[n].T.reshape(KC, 128, L).transpose(1, 0, 2)
        xTs.append(np.ascontiguousarray(xt, dtype=np.float16))

    def wslice(W, fs):
        # [128, KC, F]: [p, kc, f] with hidden = kc*128+p
        return np.ascontiguousarray(
            W[fs:fs + F, :].T.reshape(KC, 128, F).transpose(1, 0, 2),
            dtype=np.float16,
        )

    in_maps = []
    for c in range(NCORES):
        n = c // HPC
        hg = c % HPC
        fs = F * hg
        wo_d = Wo[:, fs:fs + F].T.reshape(2, 128, HIDDEN).transpose(1, 0, 2)
        in_maps.append(
            {
                "xT": xTs[n],
                "wq": wslice(Wq, fs),
                "wk": wslice(Wk, fs),
                "wv": wslice(Wv, fs),
                "wo": to_bf16_np(wo_d),
                "bq": np.ascontiguousarray(bq[fs:fs + F].reshape(2, 128).T),
                "bk": np.ascontiguousarray(bk[fs:fs + F].reshape(2, 128).T),
                "bv": np.ascontiguousarray(bv[fs:fs + F].reshape(1, F),
                                           dtype=np.float32),
                "sel": _sel_matrix(),
            }
        )
    return in_maps


def kernel(x, Wq, bq, Wk, bk, Wv, bv, Wo, bo):
    from concourse.bass_utils import run_bass_kernel_spmd

    x = np.asarray(x, dtype=np.float32)
    Wq = np.asarray(Wq, dtype=np.float32)
    Wk = np.asarray(Wk, dtype=np.float32)
    Wv = np.asarray(Wv, dtype=np.float32)
    Wo = np.asarray(Wo, dtype=np.float32)
    bq = np.asarray(bq, dtype=np.float32)
    bk = np.asarray(bk, dtype=np.float32)
    bv = np.asarray(bv, dtype=np.float32)
    bo = np.asarray(bo, dtype=np.float32)

    in_maps = _make_in_maps(x, Wq, bq, Wk, bk, Wv, bv, Wo)
    nc = _get_nc()
    res = run_bass_kernel_spmd(nc, in_maps, core_ids=list(range(NCORES)))

    out = np.zeros((NB, L, HIDDEN), dtype=np.float32)
    for c in range(NCORES):
        n = c // HPC
        p = res.results[c]["po"]  # [128, TC, HIDDEN]
        out[n] += p.transpose(1, 0, 2).reshape(L, HIDDEN)
    out += bo
    return out


def _compile_check():
    import tempfile
    from concourse.bass_utils import compile_bass_kernel

    nc = _build()
    td = tempfile.mkdtemp()
    neff = compile_bass_kernel(nc, td)
    print("COMPILE OK:", neff)


if __name__ == "__main__":
    _compile_check()
